# revision 1
# baseline (speedup 1.0000x reference)
"""Trainium2 Bass kernel for nn_CrossAttention (B=8, C=256, H=W=64).

Per-batch cross attention:
    attn[n, m] = softmax_m( sum_c h[c,n] * xs[c,m] )
    out[c, n]  = sum_m ys[c,m] * attn[n,m]

Sharding: data-parallel over batch B=8 -> one batch element per NeuronCore.

Per-core algorithm (matmuls in float32r = full-rate fp32 PE mode):
  - Phase 1: S[n_block=128, m] = h[:, n_block].T @ xs, contracting C=256
    in two PSUM accumulation steps, m in chunks of 512.  Each chunk is
    copied PSUM->SBUF on the scalar engine while DVE tracks the row max.
  - Softmax needs a true per-row max: on this dataset the logits span
    [-294, +246] while per-row maxima go down to 46, so no constant
    shift fits inside fp32's exp range.  exp runs on the scalar engine
    with bias = -rowmax (per-partition) and accum_out giving the row
    sums for free.
  - P blocks are PE-transposed (128x128) so the contraction dim m lands
    on partitions, then phase 2 accumulates acc[n,c] += P^T.T @ ysT
    over all 32 m-blocks in a single PSUM bank.
  - Normalize with DVE reciprocal + per-partition tensor_scalar, then
    PE-transpose [n, c] -> [c, n] and DMA out.
  - The n-block loop is software-pipelined: phase 1 of block nb is
    emitted before the transpose/phase-2/store tail of block nb-1 so
    the tensor engine never waits for the softmax round trip.
"""

import sys

sys.path.insert(0, "/opt/trn_rl_repo")

import numpy as np

import concourse.mybir as mybir
import concourse.tile as tile
from concourse import bacc
from concourse.bass_utils import run_bass_kernel_spmd
from concourse.masks import make_identity

B, C, H, W = 8, 256, 64, 64
N = H * W            # 4096 query positions (and support positions)
P = 128              # partitions
KC = C // P          # 2 contraction chunks over channels
NB = N // P          # 32 n-blocks of 128
MS = N // 512        # 8 m-chunks of 512
MB = N // P          # 32 m-blocks of 128 (phase 2)

F32 = mybir.dt.float32
F32R = mybir.dt.float32r
EXP = mybir.ActivationFunctionType.Exp
COPY = mybir.ActivationFunctionType.Copy


def build_nc(reps: int = 1, dma_per_rep: bool = True):
    nc = bacc.Bacc(None, target_bir_lowering=False, debug=False)

    hD = nc.dram_tensor("h", [C, N], F32, kind="ExternalInput").ap()
    xD = nc.dram_tensor("x", [C, N], F32, kind="ExternalInput").ap()
    yD = nc.dram_tensor("y", [C, N], F32, kind="ExternalInput").ap()
    oD = nc.dram_tensor("o", [C, N], F32, kind="ExternalOutput").ap()

    with tile.TileContext(nc) as tc:
        with (
            tc.tile_pool(name="consts", bufs=1) as consts,
            tc.tile_pool(name="ins", bufs=1) as in_pool,
            tc.tile_pool(name="yfch", bufs=4) as yfch_pool,
            tc.tile_pool(name="yft", bufs=1) as yft_pool,
            tc.tile_pool(name="schunk", bufs=1) as s_pool,
            tc.tile_pool(name="pchunk", bufs=2) as p_pool,
            tc.tile_pool(name="pt", bufs=3) as pt_pool,
            tc.tile_pool(name="fin", bufs=4) as fin_pool,
            tc.tile_pool(name="outs", bufs=2) as out_pool,
            tc.tile_pool(name="ps_s", bufs=3, space="PSUM") as ps_s,
            tc.tile_pool(name="ps_tr", bufs=3, space="PSUM") as ps_tr,
            tc.tile_pool(name="ps_a", bufs=2, space="PSUM") as ps_a,
        ):
            ident = consts.tile([P, P], F32)
            make_identity(nc, ident[:])
            identr = consts.tile([P, P], F32R)
            nc.vector.tensor_copy(identr[:], ident[:])

            loaded = False
            for rep in range(reps):
                do_load = dma_per_rep or not loaded
                # ---- input loads ----
                if do_load:
                    hf = [[in_pool.tile([P, 512], F32R, tag=f"hf{kc}_{g}", name=f"hf{kc}_{g}")
                           for g in range(MS)] for kc in range(KC)]
                    xf = [[in_pool.tile([P, 512], F32R, tag=f"xf{kc}_{ms}", name=f"xf{kc}_{ms}")
                           for ms in range(MS)] for kc in range(KC)]
                    for kc in range(KC):
                        for g in range(MS):
                            nc.sync.dma_start(
                                hf[kc][g][:],
                                hD[kc * P:(kc + 1) * P, g * 512:(g + 1) * 512].bitcast(F32R))
                            nc.sync.dma_start(
                                xf[kc][g][:],
                                xD[kc * P:(kc + 1) * P, g * 512:(g + 1) * 512].bitcast(F32R))

                # ---- build ysT [m, c] tiles (transient ys chunks) ----
                if do_load:
                    yft = [yft_pool.tile([P, 256], F32R, tag=f"yft{mb}", name=f"yft{mb}")
                           for mb in range(MB)]
                    for ch in range(KC):
                        for mg in range(MS):
                            yc = yfch_pool.tile([P, 512], F32, tag="yfch", name="yfch")
                            nc.sync.dma_start(
                                yc[:], yD[ch * P:(ch + 1) * P, mg * 512:(mg + 1) * 512])
                            tr4 = ps_tr.tile([P, 512], F32, tag="tr")
                            for j in range(4):
                                nc.tensor.transpose(
                                    tr4[:, j * P:(j + 1) * P], yc[:, j * P:(j + 1) * P],
                                    ident[:])
                            for j in range(4):
                                nc.vector.tensor_copy(
                                    yft[mg * 4 + j][:, ch * P:(ch + 1) * P],
                                    tr4[:, j * P:(j + 1) * P])
                    loaded = True

                # ---- pipelined main loop over n-blocks ----
                out_sb = {}
                tail_work = None

                def make_tail(nb_, p_chunks_, rec_):
                    """Tail of n-block nb_ as a list of small step closures so
                    it can be interleaved with the next block's phase 1."""
                    g_, r_ = nb_ // 4, nb_ % 4
                    state = {}

                    def setup():
                        if r_ == 0:
                            for ch in range(KC):
                                out_sb[ch] = out_pool.tile(
                                    [P, 512], F32, tag=f"osb{ch}", name=f"osb{ch}")
                        state["acc"] = ps_a.tile([P, 256], F32, tag="acc", name="acc")

                    def group(g2):
                        def run():
                            tr4 = ps_tr.tile([P, 512], F32R, tag="tr")
                            for j in range(4):
                                nc.tensor.transpose(
                                    tr4[:, j * P:(j + 1) * P],
                                    p_chunks_[g2][:, j * P:(j + 1) * P], identr[:])
                            pts = pt_pool.tile([P, 512], F32R, tag="pt")
                            nc.vector.tensor_copy(pts[:], tr4[:])
                            for j in range(4):
                                mb = g2 * 4 + j
                                nc.tensor.matmul(
                                    state["acc"][:], pts[:, j * P:(j + 1) * P],
                                    yft[mb][:],
                                    start=(mb == 0), stop=(mb == MB - 1))
                        return run

                    def finish():
                        xx = fin_pool.tile([P, 256], F32, tag="xx")
                        nc.vector.tensor_scalar_mul(xx[:], state["acc"][:], rec_[:])
                        tro = ps_tr.tile([P, 512], F32, tag="tr")
                        for ch in range(KC):
                            nc.tensor.transpose(
                                tro[:, ch * P:(ch + 1) * P],
                                xx[:, ch * P:(ch + 1) * P], ident[:])
                        for ch in range(KC):
                            nc.vector.tensor_copy(
                                out_sb[ch][:, r_ * P:(r_ + 1) * P],
                                tro[:, ch * P:(ch + 1) * P])
                        if r_ == 3:
                            for ch in range(KC):
                                nc.sync.dma_start(
                                    oD[ch * P:(ch + 1) * P, g_ * 512:(g_ + 1) * 512],
                                    out_sb[ch][:])

                    return [setup] + [group(g2) for g2 in range(MS)] + [finish]

                tail_steps = []
                for nb in range(NB):
                    g, r = nb // 4, nb % 4
                    # phase 1: S[n_block, m] in chunks of 512, interleaved
                    # with the previous block's transpose/phase-2 steps
                    rmx = fin_pool.tile([P, MS], F32, tag="rmx")
                    s_chunks = []
                    for ms in range(MS):
                        ps = ps_s.tile([P, 512], F32, tag="ps")
                        for kc in range(KC):
                            nc.tensor.matmul(
                                ps[:], hf[kc][g][:, r * P:(r + 1) * P], xf[kc][ms][:],
                                start=(kc == 0), stop=(kc == KC - 1))
                        ssb = s_pool.tile([P, 512], F32, tag=f"s{ms}", name=f"s{ms}")
                        nc.scalar.activation(ssb[:], ps[:], COPY)
                        nc.vector.reduce_max(
                            rmx[:, ms:ms + 1], ssb[:], axis=mybir.AxisListType.X)
                        s_chunks.append(ssb)
                        if tail_steps:
                            tail_steps.pop(0)()
                    nbias = fin_pool.tile([P, 1], F32, tag="nbias")
                    nc.vector.reduce_max(
                        nbias[:], rmx[:], axis=mybir.AxisListType.X, negate=True)
                    rsum = fin_pool.tile([P, MS], F32, tag="rsum")
                    p_chunks = []
                    for ms in range(MS):
                        pch = p_pool.tile([P, 512], F32R, tag=f"p{ms}", name=f"p{ms}")
                        nc.scalar.activation(
                            pch[:], s_chunks[ms][:], EXP, bias=nbias[:],
                            accum_out=rsum[:, ms:ms + 1])
                        p_chunks.append(pch)
                        if tail_steps:
                            tail_steps.pop(0)()
                    rs1 = fin_pool.tile([P, 1], F32, tag="rs1")
                    nc.vector.reduce_sum(rs1[:], rsum[:], axis=mybir.AxisListType.X)
                    rec = fin_pool.tile([P, 1], F32, tag="rec")
                    nc.vector.reciprocal(rec[:], rs1[:])

                    while tail_steps:
                        tail_steps.pop(0)()
                    tail_steps = make_tail(nb, p_chunks, rec)
                while tail_steps:
                    tail_steps.pop(0)()

    nc.finalize()
    return nc


_cache = {}


def _get_nc(reps: int = 1, dma_per_rep: bool = True):
    key = (reps, dma_per_rep)
    if key not in _cache:
        _cache[key] = build_nc(reps, dma_per_rep)
    return _cache[key]


def kernel(h: np.ndarray, xs: np.ndarray, ys: np.ndarray) -> np.ndarray:
    assert h.shape == (B, C, H, W) and xs.shape == (B, C, H, W)
    nc = _get_nc(1)
    in_maps = []
    for b in range(B):
        in_maps.append({
            "h": np.ascontiguousarray(h[b], dtype=np.float32).reshape(C, N),
            "x": np.ascontiguousarray(xs[b], dtype=np.float32).reshape(C, N),
            "y": np.ascontiguousarray(ys[b], dtype=np.float32).reshape(C, N),
        })
    res = run_bass_kernel_spmd(nc, in_maps, list(range(B)))
    out = np.stack([res.results[b]["o"] for b in range(B)], axis=0)
    return out.reshape(B, C, H, W).astype(np.float32)

